# revision 25
# baseline (speedup 1.0000x reference)
"""Coordinate-descent (alternating Gauss-Seidel) kernel for Trainium2.

B=4 factorizations x ~ u @ v^T, M=N=4096, R=32.

The R-step Gauss-Seidel sweep equals a triangular solve:
  u_new = (a1 + eps - u_old @ L1) @ inv(triu(b1) + eps*I),  L1 = tril(b1,-1)
computed here entirely in transposed space:
  u_new^T = U1inv^T @ (a1^T + eps - L1^T @ u_old^T)
so a1 is consumed straight from the PE burst output (no layout fixups) and
each solve is two matmuls. u-side factors are host-precomputed from v;
v-side factors come from b2 = u_new^T u_new, inverted exactly on-chip via
the nilpotent identity (I+M)^-1 = (I-M)(I+M^2)(I+M^4)(I+M^8)(I+M^16).

Cross-sharding removes all on-chip transposes of x and the big a2-partial
ReduceScatter: core c computes a1/u_new for m-rows [c*512,(c+1)*512)
(consuming a host-pretransposed xT column slice), AllGathers the tiny
u_new (32KB bf16), then computes a2/v_new for n-rows [c*512,(c+1)*512)
with the full m contraction done locally on the natural-layout column
slice of x. Two explicit phases keep the PE stream dense: all four a1
bursts + u-solves + AllGather triggers first, the whole v-side second, so
collective latency hides under later batches' compute.
"""

import os
from contextlib import ExitStack

import numpy as np
from ml_dtypes import bfloat16

import concourse.bass as bass
import concourse.tile as tile
from concourse import bacc, mybir
from concourse.bass import ds
from concourse.bass_utils import run_bass_kernel_spmd
from concourse.masks import make_identity

B, M, N, R = 4, 4096, 4096, 32
NCORES = 8
MS = M // NCORES          # 512 rows per core
MC = MS // 128            # 4 chunks of 128 within the slice
NQ = M // 128             # 32 chunks of 128 over the full dim
NGRP = 4                  # PE column groups (tile_position packing)
CHG = NQ // NGRP          # 8 contraction chunks per column group
EPS = 1e-8
FP32 = mybir.dt.float32
BF16 = mybir.dt.bfloat16
ALU = mybir.AluOpType

COLTILE = True            # 4x column-packed PE bursts

_CACHE = {}
LAST_RESULT = None


def _gram_burst(nc, out_ps, lhs_chunks, rhs_chunks):
    """Accumulate sum_q lhs[q]^T @ rhs[q] into 4 column groups of out_ps."""
    if COLTILE:
        for k in range(CHG):
            for g in range(NGRP):
                q = g * CHG + k
                nc.tensor.matmul(
                    out_ps[32 * g:32 * (g + 1), :], lhsT=lhs_chunks(q),
                    rhs=rhs_chunks(q), start=(k == 0), stop=(k == CHG - 1),
                    tile_position=(0, 32 * g), skip_group_check=True)
    else:
        for q in range(NQ):
            nc.tensor.matmul(out_ps[0:32, :], lhsT=lhs_chunks(q),
                             rhs=rhs_chunks(q), start=(q == 0),
                             stop=(q == NQ - 1))


def _group_sum(nc, wk, out_ap, ps, free, tag):
    """out = sum of the 4 [32, free] column-group partials in ps.

    One PSUM operand per DVE instruction, accumulator in SBUF.
    """
    if COLTILE:
        acc = wk.tile([32, free], FP32, tag=tag + "a")
        nc.vector.tensor_copy(acc[:], ps[0:32, :])
        nc.vector.tensor_tensor(out=acc[:], in0=acc[:], in1=ps[32:64, :],
                                op=ALU.add)
        nc.vector.tensor_tensor(out=acc[:], in0=acc[:], in1=ps[64:96, :],
                                op=ALU.add)
        nc.vector.tensor_tensor(out=out_ap, in0=acc[:], in1=ps[96:128, :],
                                op=ALU.add)
    else:
        nc.vector.tensor_copy(out_ap, ps[0:32, :])


def _build():
    nc = bacc.Bacc("TRN2", target_bir_lowering=False, debug=False,
                   num_devices=NCORES)

    xT_my = nc.dram_tensor("xT_my", [128, B * NQ * MS], BF16,
                           kind="ExternalInput").ap()
    x_cs = nc.dram_tensor("x_cs", [128, B * NQ * MS], BF16,
                          kind="ExternalInput").ap()
    v_bf = nc.dram_tensor("v_bf", [128, B * NQ * R], BF16,
                          kind="ExternalInput").ap()
    u_myT = nc.dram_tensor("u_myT", [R, B * MS], FP32,
                           kind="ExternalInput").ap()
    v_myT = nc.dram_tensor("v_myT", [R, B * MS], FP32,
                           kind="ExternalInput").ap()
    L1_d = nc.dram_tensor("L1", [B, R, R], FP32, kind="ExternalInput").ap()
    U1inv_d = nc.dram_tensor("U1inv", [B, R, R], FP32,
                             kind="ExternalInput").ap()
    I32_d = nc.dram_tensor("I32", [R, R], FP32, kind="ExternalInput").ap()
    UM_d = nc.dram_tensor("UM", [R, R], FP32, kind="ExternalInput").ap()
    LM_d = nc.dram_tensor("LM", [R, R], FP32, kind="ExternalInput").ap()
    ONES_d = nc.dram_tensor("ONES", [R, 1], FP32, kind="ExternalInput").ap()
    u_out = nc.dram_tensor("u_out", [128, B * MC * R], FP32,
                           kind="ExternalOutput").ap()
    v_out = nc.dram_tensor("v_out", [128, B * MC * R], FP32,
                           kind="ExternalOutput").ap()

    # AllGather buffers, partition-major within each rank block so every
    # DMA descriptor is a 256B contiguous run.
    ag_in = nc.dram_tensor("ag_in", [B * MS * R], BF16)
    ag_outA = nc.dram_tensor("ag_outA", [NCORES * 2 * MS * R], BF16,
                             addr_space="Shared")
    ag_outB = nc.dram_tensor("ag_outB", [NCORES * 2 * MS * R], BF16,
                             addr_space="Shared")

    with tile.TileContext(nc) as tc, ExitStack() as ctx:
        const = ctx.enter_context(tc.tile_pool(name="const", bufs=1))
        xp = ctx.enter_context(tc.tile_pool(name="xp", bufs=4))
        big = ctx.enter_context(tc.tile_pool(name="big", bufs=1))
        ubp = ctx.enter_context(tc.tile_pool(name="ubp", bufs=4))
        wk = ctx.enter_context(tc.tile_pool(name="wk", bufs=2))
        sm = ctx.enter_context(tc.tile_pool(name="sm", bufs=2))
        pap = ctx.enter_context(tc.tile_pool(name="pap", bufs=2,
                                             space="PSUM"))
        pst = ctx.enter_context(tc.tile_pool(name="pst", bufs=2,
                                             space="PSUM"))
        ptp = ctx.enter_context(tc.tile_pool(name="ptp", bufs=2,
                                             space="PSUM"))
        pnp = ctx.enter_context(tc.tile_pool(name="pnp", bufs=1,
                                             space="PSUM"))
        pb2p = ctx.enter_context(tc.tile_pool(name="pb2p", bufs=1,
                                              space="PSUM"))

        xT0_t = xp.tile([128, NQ, MS], BF16, tag="x")
        half0 = NQ * MS // 2
        nc.sync.dma_start(
            xT0_t[:, 0:NQ // 2, :].rearrange("p q m -> p (q m)"),
            xT_my[:, ds(0, half0)])
        nc.sync.dma_start(
            xT0_t[:, NQ // 2:NQ, :].rearrange("p q m -> p (q m)"),
            xT_my[:, ds(half0, half0)])

        ident_f = const.tile([128, 128], FP32)
        make_identity(nc, ident_f)
        I32_t = const.tile([R, R], FP32)
        nc.scalar.dma_start(I32_t[:], I32_d)
        UM_t = const.tile([R, R], FP32)
        nc.scalar.dma_start(UM_t[:], UM_d)
        LM_t = const.tile([R, R], FP32)
        nc.scalar.dma_start(LM_t[:], LM_d)
        ONES_t = const.tile([R, 1], FP32)
        nc.scalar.dma_start(ONES_t[:], ONES_d)
        L1_ts, U1_ts = [], []
        for b in range(B):
            t = const.tile([R, R], FP32, name=f"L1_{b}")
            nc.scalar.dma_start(t[:], L1_d[b])
            L1_ts.append(t)
            t = const.tile([R, R], FP32, name=f"U1_{b}")
            nc.scalar.dma_start(t[:], U1inv_d[b])
            U1_ts.append(t)

        vb_t = big.tile([128, B, NQ, R], BF16)
        nc.scalar.dma_start(vb_t[:].rearrange("p b q r -> p (b q r)"), v_bf)
        u_myT_t = big.tile([R, B, MS], FP32)
        nc.scalar.dma_start(u_myT_t[:].rearrange("r b m -> r (b m)"), u_myT)
        v_myT_t = big.tile([R, B, MS], FP32)
        nc.scalar.dma_start(v_myT_t[:].rearrange("r b m -> r (b m)"), v_myT)

        # =========== phase 1: a1 bursts, u-solves (pipeline-shifted) ======
        p1_state = {}
        p1_unew = {}

        def p1_burst(b):
            if b == 0:
                xT_t = xT0_t
            else:
                xT_t = xp.tile([128, NQ, MS], BF16, tag="x")
                half = NQ * MS // 2
                nc.sync.dma_start(
                    xT_t[:, 0:NQ // 2, :].rearrange("p q m -> p (q m)"),
                    xT_my[:, ds(b * NQ * MS, half)])
                nc.sync.dma_start(
                    xT_t[:, NQ // 2:NQ, :].rearrange("p q m -> p (q m)"),
                    xT_my[:, ds(b * NQ * MS + half, half)])
            pa1 = pap.tile([128, MS], FP32, tag="pa")
            _gram_burst(nc, pa1,
                        lambda q: vb_t[:, b, q, :],
                        lambda q: xT_t[:, q, :])
            p1_state[b] = pa1

        def p1_solve(b):
            pa1 = p1_state.pop(b)
            a1T_sb = wk.tile([32, MS], FP32, tag="a1T")
            _group_sum(nc, wk, a1T_sb[:], pa1, MS, "g1")
            # u_new^T = U1inv^T @ (a1^T + eps - L1^T @ u_old^T)
            pLu = pst.tile([32, MS], FP32, tag="pq")
            nc.tensor.matmul(pLu[:], lhsT=L1_ts[b][:], rhs=u_myT_t[:, b, :],
                             start=True, stop=True)
            RHST_sb = wk.tile([32, MS], FP32, tag="RT")
            nc.vector.scalar_tensor_tensor(out=RHST_sb[:], in0=a1T_sb[:],
                                           scalar=EPS, in1=pLu[:],
                                           op0=ALU.add, op1=ALU.subtract)
            pUNT = pst.tile([32, MS], FP32, tag="pq")
            nc.tensor.matmul(pUNT[:], lhsT=U1_ts[b][:], rhs=RHST_sb[:],
                             start=True, stop=True)
            uNT_sb = wk.tile([32, MS], FP32, tag="uNT")
            nc.vector.tensor_copy(uNT_sb[:], pUNT[:])
            pUN = ptp.tile([128, MC, R], FP32, tag="pt")
            for i in range(MC):
                nc.tensor.transpose(pUN[:, i],
                                    uNT_sb[:, i * 128:(i + 1) * 128],
                                    ident_f[:R, :R])
            u_new_bf = sm.tile([128, MC, R], BF16, tag="unb")
            nc.vector.tensor_copy(u_new_bf[:], pUN[:])
            dst = ag_in.ap()
            nc.scalar.dma_start(
                bass.AP(dst.tensor, dst.offset + b * MS * R,
                        [[MC * R, 128], [R, MC], [1, R]]),
                u_new_bf[:])
            u_new_sb = sm.tile([128, MC, R], FP32, tag="un", bufs=4)
            nc.vector.tensor_copy(u_new_sb[:], pUN[:])
            p1_unew[b] = u_new_sb

        for b in range(B):
            p1_burst(b)
            if b > 0:
                p1_solve(b - 1)
            if b == 2:
                nc.gpsimd.collective_compute(
                    "AllGather", ALU.bypass,
                    replica_groups=[list(range(NCORES))],
                    ins=[ag_in.ap()[ds(0, 2 * MS * R)]],
                    outs=[ag_outA.ap()])
        p1_solve(B - 1)
        nc.gpsimd.collective_compute(
            "AllGather", ALU.bypass,
            replica_groups=[list(range(NCORES))],
            ins=[ag_in.ap()[ds(2 * MS * R, 2 * MS * R)]],
            outs=[ag_outB.ap()])

        # =========== phase 2: b2+inverse first, then a2 bursts + v-solves =
        p2_ub = {}
        p2_inv = {}
        p2_state = {}

        def p2_pre(b):
            ub_t = ubp.tile([128, NCORES, MC, R], BF16, tag="ub")
            ago = (ag_outA if b < 2 else ag_outB).ap()
            nc.scalar.dma_start(
                ub_t[:],
                bass.AP(ago.tensor, ago.offset + (b % 2) * MS * R,
                        [[MC * R, 128], [2 * MS * R, NCORES], [R, MC],
                         [1, R]]))
            ub_q = ub_t[:].rearrange("p c i r -> p (c i) r")
            p2_ub[b] = ub_q
            pb2 = pb2p.tile([128, R], FP32, tag="pb2")
            _gram_burst(nc, pb2,
                        lambda q: ub_q[:, q, :],
                        lambda q: ub_q[:, q, :])
            b2_sb = sm.tile([R, R], FP32, tag="b2")
            _group_sum(nc, wk, b2_sb[:], pb2, R, "g3")

            # ---- exact inv(triu(b2)+eps I), nilpotent squaring -----------
            junk = sm.tile([R, R], FP32, tag="junk")
            nc.vector.tensor_tensor(out=junk[:], in0=b2_sb[:], in1=I32_t[:],
                                    op=ALU.mult)
            pdc = pnp.tile([R, R], FP32, tag="pn")
            nc.tensor.matmul(pdc[:, 0:1], lhsT=junk[:], rhs=ONES_t[:],
                             start=True, stop=True)
            dcol = sm.tile([R, 1], FP32, tag="dcol")
            nc.vector.tensor_scalar_add(dcol[:], pdc[:, 0:1], EPS)
            rd = sm.tile([R, 1], FP32, tag="rd")
            nc.vector.reciprocal(rd[:], dcol[:])
            Mm = sm.tile([R, R], FP32, tag="Mm")
            nc.vector.tensor_tensor(out=Mm[:], in0=b2_sb[:], in1=UM_t[:],
                                    op=ALU.mult)
            rd_bc = bass.AP(rd[:].tensor, rd[:].offset,
                            [rd[:].ap[0], [0, R]])
            nc.vector.tensor_tensor(out=Mm[:], in0=Mm[:], in1=rd_bc,
                                    op=ALU.mult)
            L2_sb = sm.tile([R, R], FP32, tag=f"L2_{b}")
            nc.vector.tensor_tensor(out=L2_sb[:], in0=b2_sb[:], in1=LM_t[:],
                                    op=ALU.mult)

            def tr32(src_ap, tagn):
                ps = pnp.tile([R, R], FP32, tag="pn")
                nc.tensor.transpose(ps[:], src_ap, ident_f[:R, :R])
                t = sm.tile([R, R], FP32, tag=tagn)
                nc.vector.tensor_copy(t[:], ps[:])
                return t

            def mm32(lhsT_t, rhs_ap, tagn):
                ps = pnp.tile([R, R], FP32, tag="pn")
                nc.tensor.matmul(ps[:], lhsT=lhsT_t[:], rhs=rhs_ap,
                                 start=True, stop=True)
                t = sm.tile([R, R], FP32, tag=tagn)
                nc.vector.tensor_copy(t[:], ps[:])
                return t

            def add_I(src_t, tagn, sub=False):
                t = sm.tile([R, R], FP32, tag=tagn)
                if sub:
                    nc.vector.tensor_tensor(out=t[:], in0=I32_t[:],
                                            in1=src_t[:], op=ALU.subtract)
                else:
                    nc.vector.tensor_tensor(out=t[:], in0=src_t[:],
                                            in1=I32_t[:], op=ALU.add)
                return t

            Mt = tr32(Mm[:], "Mt")
            M2 = mm32(Mt, Mm[:], "M2")
            M2t = tr32(M2[:], "M2t")
            M4 = mm32(M2t, M2[:], "M4")
            M4t = tr32(M4[:], "M4t")
            M8 = mm32(M4t, M4[:], "M8")
            M8t = tr32(M8[:], "M8t")
            M16 = mm32(M8t, M8[:], "M16")
            M16t = tr32(M16[:], "M16t")
            Dinv = sm.tile([R, R], FP32, tag="Dinv")
            rd_bc2 = bass.AP(rd[:].tensor, rd[:].offset,
                             [rd[:].ap[0], [0, R]])
            nc.vector.tensor_tensor(out=Dinv[:], in0=I32_t[:], in1=rd_bc2,
                                    op=ALU.mult)
            T1 = mm32(add_I(M16t, "G4t"), Dinv[:], "T1")
            T2 = mm32(add_I(M8t, "G3t"), T1[:], "T2")
            T3 = mm32(add_I(M4t, "G2t"), T2[:], "T3")
            T4 = mm32(add_I(M2t, "G1t"), T3[:], "T4")
            U2inv = mm32(add_I(Mt, "G0t", sub=True), T4[:], f"U2_{b}")
            p2_inv[b] = (L2_sb, U2inv)

        def p2_burst(b):
            u_new_sb = p1_unew.pop(b)
            nc.scalar.dma_start(u_out[:, ds(b * MC * R, MC * R)],
                                u_new_sb[:].rearrange("p i r -> p (i r)"))
            xc_t = xp.tile([128, NQ, MS], BF16, tag="x")
            nc.sync.dma_start(xc_t[:].rearrange("p q m -> p (q m)"),
                              x_cs[:, ds(b * NQ * MS, NQ * MS)])
            ub_q = p2_ub[b]
            pa2 = pap.tile([128, MS], FP32, tag="pa")
            _gram_burst(nc, pa2,
                        lambda q: ub_q[:, q, :],
                        lambda q: xc_t[:, q, :])
            p2_state[b] = pa2

        def p2_solve(b):
            pa2 = p2_state.pop(b)
            L2_sb, U2inv = p2_inv.pop(b)
            a2T_sb = wk.tile([32, MS], FP32, tag="a2T")
            _group_sum(nc, wk, a2T_sb[:], pa2, MS, "g2")
            pLv = pst.tile([32, MS], FP32, tag="pq")
            nc.tensor.matmul(pLv[:], lhsT=L2_sb[:], rhs=v_myT_t[:, b, :],
                             start=True, stop=True)
            RHS2T_sb = wk.tile([32, MS], FP32, tag="R2T")
            nc.vector.scalar_tensor_tensor(out=RHS2T_sb[:], in0=a2T_sb[:],
                                           scalar=EPS, in1=pLv[:],
                                           op0=ALU.add, op1=ALU.subtract)
            pVNT = pst.tile([32, MS], FP32, tag="pq")
            nc.tensor.matmul(pVNT[:], lhsT=U2inv[:], rhs=RHS2T_sb[:],
                             start=True, stop=True)
            vNT_sb = wk.tile([32, MS], FP32, tag="vNT")
            nc.vector.tensor_copy(vNT_sb[:], pVNT[:])
            pVN = ptp.tile([128, MC, R], FP32, tag="pt")
            for i in range(MC):
                nc.tensor.transpose(pVN[:, i],
                                    vNT_sb[:, i * 128:(i + 1) * 128],
                                    ident_f[:R, :R])
            v_new_sb = sm.tile([128, MC, R], FP32, tag="vn")
            nc.vector.tensor_copy(v_new_sb[:], pVN[:])
            nc.scalar.dma_start(v_out[:, ds(b * MC * R, MC * R)],
                                v_new_sb[:].rearrange("p i r -> p (i r)"))

        for b in range(B):
            p2_pre(b)
        for b in range(B):
            p2_burst(b)
            if b > 0:
                p2_solve(b - 1)
        p2_solve(B - 1)

    nc.compile()
    return nc


def _prep_inputs(x, u, v):
    """Host-side layout/precompute. Returns per-core in_maps."""
    x = np.ascontiguousarray(x, dtype=np.float32)
    u = np.ascontiguousarray(u, dtype=np.float32)
    v = np.ascontiguousarray(v, dtype=np.float32)

    xb = x.astype(bfloat16)
    # xT slice per core: [c, p, b, q, m_l] with n = q*128+p, m = c*512+m_l
    xT_all = np.ascontiguousarray(
        xb.reshape(B, NCORES, MS, NQ, 128).transpose(1, 4, 0, 3, 2)
    ).reshape(NCORES, 128, B * NQ * MS)
    # natural column slice per core: [c, p, b, i, n_l], m = i*128+p
    xcs_all = np.ascontiguousarray(
        xb.reshape(B, NQ, 128, NCORES, MS).transpose(3, 2, 0, 1, 4)
    ).reshape(NCORES, 128, B * NQ * MS)

    v_bf = np.ascontiguousarray(
        v.astype(bfloat16).reshape(B, NQ, 128, R).transpose(2, 0, 1, 3)
    ).reshape(128, B * NQ * R)
    # transposed factor slices: [c, r, b, m_l]
    uT_all = np.ascontiguousarray(
        u.reshape(B, NCORES, MS, R).transpose(1, 3, 0, 2)
    ).reshape(NCORES, R, B * MS)
    vT_all = np.ascontiguousarray(
        v.reshape(B, NCORES, MS, R).transpose(1, 3, 0, 2)
    ).reshape(NCORES, R, B * MS)

    v64 = v.astype(np.float64)
    b1 = np.einsum('bnr,bns->brs', v64, v64)
    L1 = np.tril(b1, -1).astype(np.float32)
    U1inv = np.stack([
        np.linalg.inv(np.triu(b1[b]) + EPS * np.eye(R)) for b in range(B)
    ]).astype(np.float32)

    I32 = np.eye(R, dtype=np.float32)
    UM = np.triu(np.ones((R, R), dtype=np.float32), 1)
    LM = np.tril(np.ones((R, R), dtype=np.float32), -1)
    ONES = np.ones((R, 1), dtype=np.float32)

    in_maps = []
    for c in range(NCORES):
        in_maps.append({
            "xT_my": xT_all[c],
            "x_cs": xcs_all[c],
            "v_bf": v_bf,
            "u_myT": uT_all[c],
            "v_myT": vT_all[c],
            "L1": L1,
            "U1inv": U1inv,
            "I32": I32,
            "UM": UM,
            "LM": LM,
            "ONES": ONES,
        })
    return in_maps


def kernel(x, u, v):
    global LAST_RESULT
    if "nc" not in _CACHE:
        _CACHE["nc"] = _build()
    nc = _CACHE["nc"]

    in_maps = _prep_inputs(x, u, v)
    res = run_bass_kernel_spmd(nc, in_maps, list(range(NCORES)),
                               trace=os.environ.get("KBENCH_TRACE") == "1")
    LAST_RESULT = res

    def assemble(key):
        arr = np.stack([res.results[c][key] for c in range(NCORES)])
        return np.ascontiguousarray(
            arr.reshape(NCORES, 128, B, MC, R).transpose(2, 0, 3, 1, 4)
        ).reshape(B, M, R)

    return (assemble("u_out"), assemble("v_out"))


# revision 27
# speedup vs baseline: 1.1358x; 1.1358x over previous
"""Coordinate-descent (alternating Gauss-Seidel) kernel for Trainium2.

B=4 factorizations x ~ u @ v^T, M=N=4096, R=32.

The R-step Gauss-Seidel sweep equals a triangular solve:
  u_new = (a1 + eps - u_old @ L1) @ inv(triu(b1) + eps*I),  L1 = tril(b1,-1)
computed here entirely in transposed space:
  u_new^T = U1inv^T @ (a1^T + eps - L1^T @ u_old^T)
so a1 is consumed straight from the PE burst output (no layout fixups) and
each solve is two matmuls. u-side factors are host-precomputed from v;
v-side factors come from b2 = u_new^T u_new, inverted exactly on-chip via
the nilpotent identity (I+M)^-1 = (I-M)(I+M^2)(I+M^4)(I+M^8)(I+M^16).

Cross-sharding removes all on-chip transposes of x and the big a2-partial
ReduceScatter: core c computes a1/u_new for m-rows [c*512,(c+1)*512)
(consuming a host-pretransposed xT column slice), AllGathers the tiny
u_new (32KB bf16), then computes a2/v_new for n-rows [c*512,(c+1)*512)
with the full m contraction done locally on the natural-layout column
slice of x. Two explicit phases keep the PE stream dense: all four a1
bursts + u-solves + AllGather triggers first, the whole v-side second, so
collective latency hides under later batches' compute.
"""

import os
from contextlib import ExitStack

import numpy as np
from ml_dtypes import bfloat16

import concourse.bass as bass
import concourse.tile as tile
from concourse import bacc, mybir
from concourse.bass import ds
from concourse.bass_utils import run_bass_kernel_spmd
from concourse.masks import make_identity

B, M, N, R = 4, 4096, 4096, 32
NCORES = 8
MS = M // NCORES          # 512 rows per core
MC = MS // 128            # 4 chunks of 128 within the slice
NQ = M // 128             # 32 chunks of 128 over the full dim
NGRP = 4                  # PE column groups (tile_position packing)
CHG = NQ // NGRP          # 8 contraction chunks per column group
EPS = 1e-8
FP32 = mybir.dt.float32
BF16 = mybir.dt.bfloat16
ALU = mybir.AluOpType

COLTILE = True            # 4x column-packed PE bursts

_CACHE = {}
LAST_RESULT = None


def _chunk_of(g, k):
    """Group g's k-th contraction chunk; first 4 from the low half of the
    chunk range, last 4 from the high half (so bursts start on half data)."""
    return 4 * g + k if k < 4 else NQ // 2 + 4 * g + (k - 4)


def _gram_burst(nc, out_ps, lhs_chunks, rhs_chunks):
    """Accumulate sum_q lhs[q]^T @ rhs[q] into 4 column groups of out_ps."""
    if COLTILE:
        for k in range(CHG):
            for g in range(NGRP):
                q = _chunk_of(g, k)
                nc.tensor.matmul(
                    out_ps[32 * g:32 * (g + 1), :], lhsT=lhs_chunks(q),
                    rhs=rhs_chunks(q), start=(k == 0), stop=(k == CHG - 1),
                    tile_position=(0, 32 * g), skip_group_check=True)
    else:
        for q in range(NQ):
            nc.tensor.matmul(out_ps[0:32, :], lhsT=lhs_chunks(q),
                             rhs=rhs_chunks(q), start=(q == 0),
                             stop=(q == NQ - 1))


def _group_sum(nc, wk, out_ap, ps, free, tag):
    """out = sum of the 4 [32, free] column-group partials in ps.

    One PSUM operand per DVE instruction, accumulator in SBUF.
    """
    if COLTILE:
        acc = wk.tile([32, free], FP32, tag=tag + "a")
        nc.vector.tensor_copy(acc[:], ps[0:32, :])
        nc.vector.tensor_tensor(out=acc[:], in0=acc[:], in1=ps[32:64, :],
                                op=ALU.add)
        nc.vector.tensor_tensor(out=acc[:], in0=acc[:], in1=ps[64:96, :],
                                op=ALU.add)
        nc.vector.tensor_tensor(out=out_ap, in0=acc[:], in1=ps[96:128, :],
                                op=ALU.add)
    else:
        nc.vector.tensor_copy(out_ap, ps[0:32, :])


def _build():
    nc = bacc.Bacc("TRN2", target_bir_lowering=False, debug=False,
                   num_devices=NCORES)

    xT_my = nc.dram_tensor("xT_my", [128, B * NQ * MS], BF16,
                           kind="ExternalInput").ap()
    x_cs = nc.dram_tensor("x_cs", [128, B * NQ * MS], BF16,
                          kind="ExternalInput").ap()
    v_bf = nc.dram_tensor("v_bf", [128, B * NQ * R], BF16,
                          kind="ExternalInput").ap()
    u_myT = nc.dram_tensor("u_myT", [R, B * MS], FP32,
                           kind="ExternalInput").ap()
    v_myT = nc.dram_tensor("v_myT", [R, B * MS], FP32,
                           kind="ExternalInput").ap()
    L1_d = nc.dram_tensor("L1", [B, R, R], FP32, kind="ExternalInput").ap()
    U1inv_d = nc.dram_tensor("U1inv", [B, R, R], FP32,
                             kind="ExternalInput").ap()
    I32_d = nc.dram_tensor("I32", [R, R], FP32, kind="ExternalInput").ap()
    UM_d = nc.dram_tensor("UM", [R, R], FP32, kind="ExternalInput").ap()
    LM_d = nc.dram_tensor("LM", [R, R], FP32, kind="ExternalInput").ap()
    ONES_d = nc.dram_tensor("ONES", [R, 1], FP32, kind="ExternalInput").ap()
    u_out = nc.dram_tensor("u_out", [128, B * MC * R], FP32,
                           kind="ExternalOutput").ap()
    v_out = nc.dram_tensor("v_out", [128, B * MC * R], FP32,
                           kind="ExternalOutput").ap()

    # AllGather buffers, partition-major within each rank block so every
    # DMA descriptor is a 256B contiguous run.
    ag_in = nc.dram_tensor("ag_in", [B * MS * R], BF16)
    ag_outA = nc.dram_tensor("ag_outA", [NCORES * 2 * MS * R], BF16,
                             addr_space="Shared")
    ag_outB = nc.dram_tensor("ag_outB", [NCORES * 2 * MS * R], BF16,
                             addr_space="Shared")

    with tile.TileContext(nc) as tc, ExitStack() as ctx:
        const = ctx.enter_context(tc.tile_pool(name="const", bufs=1))
        xp = ctx.enter_context(tc.tile_pool(name="xp", bufs=8))
        big = ctx.enter_context(tc.tile_pool(name="big", bufs=1))
        ubp = ctx.enter_context(tc.tile_pool(name="ubp", bufs=4))
        wk = ctx.enter_context(tc.tile_pool(name="wk", bufs=2))
        sm = ctx.enter_context(tc.tile_pool(name="sm", bufs=2))
        pap = ctx.enter_context(tc.tile_pool(name="pap", bufs=2,
                                             space="PSUM"))
        pst = ctx.enter_context(tc.tile_pool(name="pst", bufs=1,
                                             space="PSUM"))
        ptp = ctx.enter_context(tc.tile_pool(name="ptp", bufs=1,
                                             space="PSUM"))
        pnp = ctx.enter_context(tc.tile_pool(name="pnp", bufs=3,
                                             space="PSUM"))
        pb2p = ctx.enter_context(tc.tile_pool(name="pb2p", bufs=1,
                                              space="PSUM"))

        half0 = NQ * MS // 2
        xT0_a = xp.tile([128, NQ // 2, MS], BF16, tag="x")
        nc.sync.dma_start(xT0_a[:].rearrange("p q m -> p (q m)"),
                          xT_my[:, ds(0, half0)])
        xT0_b = xp.tile([128, NQ // 2, MS], BF16, tag="x")
        nc.sync.dma_start(xT0_b[:].rearrange("p q m -> p (q m)"),
                          xT_my[:, ds(half0, half0)])

        ident_f = const.tile([128, 128], FP32)
        make_identity(nc, ident_f)
        I32_t = const.tile([R, R], FP32)
        nc.scalar.dma_start(I32_t[:], I32_d)
        UM_t = const.tile([R, R], FP32)
        nc.scalar.dma_start(UM_t[:], UM_d)
        LM_t = const.tile([R, R], FP32)
        nc.scalar.dma_start(LM_t[:], LM_d)
        ONES_t = const.tile([R, 1], FP32)
        nc.scalar.dma_start(ONES_t[:], ONES_d)
        L1_ts, U1_ts = [], []
        for b in range(B):
            t = const.tile([R, R], FP32, name=f"L1_{b}")
            nc.scalar.dma_start(t[:], L1_d[b])
            L1_ts.append(t)
            t = const.tile([R, R], FP32, name=f"U1_{b}")
            nc.scalar.dma_start(t[:], U1inv_d[b])
            U1_ts.append(t)

        vb_t = big.tile([128, B, NQ, R], BF16)
        nc.scalar.dma_start(vb_t[:].rearrange("p b q r -> p (b q r)"), v_bf)
        u_myT_t = big.tile([R, B, MS], FP32)
        nc.scalar.dma_start(u_myT_t[:].rearrange("r b m -> r (b m)"), u_myT)
        v_myT_t = big.tile([R, B, MS], FP32)
        nc.scalar.dma_start(v_myT_t[:].rearrange("r b m -> r (b m)"), v_myT)

        # =========== phase 1: a1 bursts, u-solves (pipeline-shifted) ======
        p1_state = {}
        p1_unew = {}

        def p1_burst(b):
            if b == 0:
                xa, xb = xT0_a, xT0_b
            else:
                half = NQ * MS // 2
                xa = xp.tile([128, NQ // 2, MS], BF16, tag="x")
                nc.sync.dma_start(xa[:].rearrange("p q m -> p (q m)"),
                                  xT_my[:, ds(b * NQ * MS, half)])
                xb = xp.tile([128, NQ // 2, MS], BF16, tag="x")
                nc.sync.dma_start(xb[:].rearrange("p q m -> p (q m)"),
                                  xT_my[:, ds(b * NQ * MS + half, half)])
            pa1 = pap.tile([128, MS], FP32, tag="pa")
            with tc.high_priority():
                _gram_burst(nc, pa1,
                            lambda q: vb_t[:, b, q, :],
                            lambda q: (xa if q < NQ // 2 else
                                       xb)[:, q % (NQ // 2), :])
            p1_state[b] = pa1

        def p1_solve(b):
            pa1 = p1_state.pop(b)
            a1T_sb = wk.tile([32, MS], FP32, tag="a1T")
            _group_sum(nc, wk, a1T_sb[:], pa1, MS, "g1")
            # u_new^T = U1inv^T @ (a1^T + eps - L1^T @ u_old^T)
            pLu = pst.tile([32, MS], FP32, tag="pq")
            nc.tensor.matmul(pLu[:], lhsT=L1_ts[b][:], rhs=u_myT_t[:, b, :],
                             start=True, stop=True)
            RHST_sb = wk.tile([32, MS], FP32, tag="RT")
            nc.vector.scalar_tensor_tensor(out=RHST_sb[:], in0=a1T_sb[:],
                                           scalar=EPS, in1=pLu[:],
                                           op0=ALU.add, op1=ALU.subtract)
            pUNT = pst.tile([32, MS], FP32, tag="pq")
            nc.tensor.matmul(pUNT[:], lhsT=U1_ts[b][:], rhs=RHST_sb[:],
                             start=True, stop=True)
            uNT_sb = wk.tile([32, MS], FP32, tag="uNT")
            nc.vector.tensor_copy(uNT_sb[:], pUNT[:])
            pUN = ptp.tile([128, MC, R], FP32, tag="pt")
            for i in range(MC):
                nc.tensor.transpose(pUN[:, i],
                                    uNT_sb[:, i * 128:(i + 1) * 128],
                                    ident_f[:R, :R])
            u_new_bf = sm.tile([128, MC, R], BF16, tag="unb")
            nc.vector.tensor_copy(u_new_bf[:], pUN[:])
            dst = ag_in.ap()
            nc.scalar.dma_start(
                bass.AP(dst.tensor, dst.offset + b * MS * R,
                        [[MC * R, 128], [R, MC], [1, R]]),
                u_new_bf[:])
            u_new_sb = sm.tile([128, MC, R], FP32, tag="un", bufs=4)
            nc.vector.tensor_copy(u_new_sb[:], pUN[:])
            p1_unew[b] = u_new_sb

        for b in range(B):
            p1_burst(b)
            if b > 0:
                p1_solve(b - 1)
            if b == 2:
                nc.gpsimd.collective_compute(
                    "AllGather", ALU.bypass,
                    replica_groups=[list(range(NCORES))],
                    ins=[ag_in.ap()[ds(0, 2 * MS * R)]],
                    outs=[ag_outA.ap()])
        p1_solve(B - 1)
        nc.gpsimd.collective_compute(
            "AllGather", ALU.bypass,
            replica_groups=[list(range(NCORES))],
            ins=[ag_in.ap()[ds(2 * MS * R, 2 * MS * R)]],
            outs=[ag_outB.ap()])

        # =========== phase 2: b2+inverse first, then a2 bursts + v-solves =
        p2_ub = {}
        p2_inv = {}
        p2_state = {}

        def p2_pre(b):
            ub_t = ubp.tile([128, NCORES, MC, R], BF16, tag="ub")
            ago = (ag_outA if b < 2 else ag_outB).ap()
            nc.scalar.dma_start(
                ub_t[:],
                bass.AP(ago.tensor, ago.offset + (b % 2) * MS * R,
                        [[MC * R, 128], [2 * MS * R, NCORES], [R, MC],
                         [1, R]]))
            ub_q = ub_t[:].rearrange("p c i r -> p (c i) r")
            p2_ub[b] = ub_q
            pb2 = pb2p.tile([128, R], FP32, tag="pb2")
            _gram_burst(nc, pb2,
                        lambda q: ub_q[:, q, :],
                        lambda q: ub_q[:, q, :])
            b2_sb = sm.tile([R, R], FP32, tag="b2")
            _group_sum(nc, wk, b2_sb[:], pb2, R, "g3")

            # ---- exact inv(triu(b2)+eps I), nilpotent squaring -----------
            junk = sm.tile([R, R], FP32, tag="junk")
            nc.vector.tensor_tensor(out=junk[:], in0=b2_sb[:], in1=I32_t[:],
                                    op=ALU.mult)
            pdc = pnp.tile([R, R], FP32, tag="pn")
            nc.tensor.matmul(pdc[:, 0:1], lhsT=junk[:], rhs=ONES_t[:],
                             start=True, stop=True)
            dcol = sm.tile([R, 1], FP32, tag="dcol")
            nc.vector.tensor_scalar_add(dcol[:], pdc[:, 0:1], EPS)
            rd = sm.tile([R, 1], FP32, tag="rd")
            nc.vector.reciprocal(rd[:], dcol[:])
            Mm = sm.tile([R, R], FP32, tag="Mm")
            nc.vector.tensor_tensor(out=Mm[:], in0=b2_sb[:], in1=UM_t[:],
                                    op=ALU.mult)
            rd_bc = bass.AP(rd[:].tensor, rd[:].offset,
                            [rd[:].ap[0], [0, R]])
            nc.vector.tensor_tensor(out=Mm[:], in0=Mm[:], in1=rd_bc,
                                    op=ALU.mult)
            L2_sb = sm.tile([R, R], FP32, tag=f"L2_{b}")
            nc.vector.tensor_tensor(out=L2_sb[:], in0=b2_sb[:], in1=LM_t[:],
                                    op=ALU.mult)

            def tr32(src_ap, tagn):
                ps = pnp.tile([R, R], FP32, tag="pn")
                nc.tensor.transpose(ps[:], src_ap, ident_f[:R, :R])
                t = sm.tile([R, R], FP32, tag=tagn)
                nc.vector.tensor_copy(t[:], ps[:])
                return t

            def mm32(lhsT_t, rhs_ap, tagn):
                ps = pnp.tile([R, R], FP32, tag="pn")
                nc.tensor.matmul(ps[:], lhsT=lhsT_t[:], rhs=rhs_ap,
                                 start=True, stop=True)
                t = sm.tile([R, R], FP32, tag=tagn)
                nc.vector.tensor_copy(t[:], ps[:])
                return t

            def add_I(src_t, tagn, sub=False):
                t = sm.tile([R, R], FP32, tag=tagn)
                if sub:
                    nc.vector.tensor_tensor(out=t[:], in0=I32_t[:],
                                            in1=src_t[:], op=ALU.subtract)
                else:
                    nc.vector.tensor_tensor(out=t[:], in0=src_t[:],
                                            in1=I32_t[:], op=ALU.add)
                return t

            Mt = tr32(Mm[:], "Mt")
            M2 = mm32(Mt, Mm[:], "M2")
            M2t = tr32(M2[:], "M2t")
            M4 = mm32(M2t, M2[:], "M4")
            M4t = tr32(M4[:], "M4t")
            M8 = mm32(M4t, M4[:], "M8")
            M8t = tr32(M8[:], "M8t")
            M16 = mm32(M8t, M8[:], "M16")
            M16t = tr32(M16[:], "M16t")
            Dinv = sm.tile([R, R], FP32, tag="Dinv")
            rd_bc2 = bass.AP(rd[:].tensor, rd[:].offset,
                             [rd[:].ap[0], [0, R]])
            nc.vector.tensor_tensor(out=Dinv[:], in0=I32_t[:], in1=rd_bc2,
                                    op=ALU.mult)
            T1 = mm32(add_I(M16t, "G4t"), Dinv[:], "T1")
            T2 = mm32(add_I(M8t, "G3t"), T1[:], "T2")
            T3 = mm32(add_I(M4t, "G2t"), T2[:], "T3")
            T4 = mm32(add_I(M2t, "G1t"), T3[:], "T4")
            U2inv = mm32(add_I(Mt, "G0t", sub=True), T4[:], f"U2_{b}")
            p2_inv[b] = (L2_sb, U2inv)

        def p2_burst(b):
            u_new_sb = p1_unew.pop(b)
            nc.scalar.dma_start(u_out[:, ds(b * MC * R, MC * R)],
                                u_new_sb[:].rearrange("p i r -> p (i r)"))
            half = NQ * MS // 2
            xca = xp.tile([128, NQ // 2, MS], BF16, tag="x")
            nc.sync.dma_start(xca[:].rearrange("p q m -> p (q m)"),
                              x_cs[:, ds(b * NQ * MS, half)])
            xcb = xp.tile([128, NQ // 2, MS], BF16, tag="x")
            nc.sync.dma_start(xcb[:].rearrange("p q m -> p (q m)"),
                              x_cs[:, ds(b * NQ * MS + half, half)])
            ub_q = p2_ub[b]
            pa2 = pap.tile([128, MS], FP32, tag="pa")
            with tc.high_priority():
                _gram_burst(nc, pa2,
                            lambda q: ub_q[:, q, :],
                            lambda q: (xca if q < NQ // 2 else
                                       xcb)[:, q % (NQ // 2), :])
            p2_state[b] = pa2

        def p2_solve(b):
            pa2 = p2_state.pop(b)
            L2_sb, U2inv = p2_inv.pop(b)
            a2T_sb = wk.tile([32, MS], FP32, tag="a2T")
            _group_sum(nc, wk, a2T_sb[:], pa2, MS, "g2")
            pLv = pst.tile([32, MS], FP32, tag="pq")
            nc.tensor.matmul(pLv[:], lhsT=L2_sb[:], rhs=v_myT_t[:, b, :],
                             start=True, stop=True)
            RHS2T_sb = wk.tile([32, MS], FP32, tag="R2T")
            nc.vector.scalar_tensor_tensor(out=RHS2T_sb[:], in0=a2T_sb[:],
                                           scalar=EPS, in1=pLv[:],
                                           op0=ALU.add, op1=ALU.subtract)
            pVNT = pst.tile([32, MS], FP32, tag="pq")
            nc.tensor.matmul(pVNT[:], lhsT=U2inv[:], rhs=RHS2T_sb[:],
                             start=True, stop=True)
            vNT_sb = wk.tile([32, MS], FP32, tag="vNT")
            nc.vector.tensor_copy(vNT_sb[:], pVNT[:])
            pVN = ptp.tile([128, MC, R], FP32, tag="pt")
            for i in range(MC):
                nc.tensor.transpose(pVN[:, i],
                                    vNT_sb[:, i * 128:(i + 1) * 128],
                                    ident_f[:R, :R])
            v_new_sb = sm.tile([128, MC, R], FP32, tag="vn")
            nc.vector.tensor_copy(v_new_sb[:], pVN[:])
            nc.scalar.dma_start(v_out[:, ds(b * MC * R, MC * R)],
                                v_new_sb[:].rearrange("p i r -> p (i r)"))

        for b in range(B):
            p2_pre(b)
        for b in range(B):
            p2_burst(b)
            if b > 0:
                p2_solve(b - 1)
        p2_solve(B - 1)

    nc.compile()
    return nc


def _prep_inputs(x, u, v):
    """Host-side layout/precompute. Returns per-core in_maps."""
    x = np.ascontiguousarray(x, dtype=np.float32)
    u = np.ascontiguousarray(u, dtype=np.float32)
    v = np.ascontiguousarray(v, dtype=np.float32)

    xb = x.astype(bfloat16)
    # xT slice per core: [c, p, b, q, m_l] with n = q*128+p, m = c*512+m_l
    xT_all = np.ascontiguousarray(
        xb.reshape(B, NCORES, MS, NQ, 128).transpose(1, 4, 0, 3, 2)
    ).reshape(NCORES, 128, B * NQ * MS)
    # natural column slice per core: [c, p, b, i, n_l], m = i*128+p
    xcs_all = np.ascontiguousarray(
        xb.reshape(B, NQ, 128, NCORES, MS).transpose(3, 2, 0, 1, 4)
    ).reshape(NCORES, 128, B * NQ * MS)

    v_bf = np.ascontiguousarray(
        v.astype(bfloat16).reshape(B, NQ, 128, R).transpose(2, 0, 1, 3)
    ).reshape(128, B * NQ * R)
    # transposed factor slices: [c, r, b, m_l]
    uT_all = np.ascontiguousarray(
        u.reshape(B, NCORES, MS, R).transpose(1, 3, 0, 2)
    ).reshape(NCORES, R, B * MS)
    vT_all = np.ascontiguousarray(
        v.reshape(B, NCORES, MS, R).transpose(1, 3, 0, 2)
    ).reshape(NCORES, R, B * MS)

    v64 = v.astype(np.float64)
    b1 = np.einsum('bnr,bns->brs', v64, v64)
    L1 = np.tril(b1, -1).astype(np.float32)
    U1inv = np.stack([
        np.linalg.inv(np.triu(b1[b]) + EPS * np.eye(R)) for b in range(B)
    ]).astype(np.float32)

    I32 = np.eye(R, dtype=np.float32)
    UM = np.triu(np.ones((R, R), dtype=np.float32), 1)
    LM = np.tril(np.ones((R, R), dtype=np.float32), -1)
    ONES = np.ones((R, 1), dtype=np.float32)

    in_maps = []
    for c in range(NCORES):
        in_maps.append({
            "xT_my": xT_all[c],
            "x_cs": xcs_all[c],
            "v_bf": v_bf,
            "u_myT": uT_all[c],
            "v_myT": vT_all[c],
            "L1": L1,
            "U1inv": U1inv,
            "I32": I32,
            "UM": UM,
            "LM": LM,
            "ONES": ONES,
        })
    return in_maps


def kernel(x, u, v):
    global LAST_RESULT
    if "nc" not in _CACHE:
        _CACHE["nc"] = _build()
    nc = _CACHE["nc"]

    in_maps = _prep_inputs(x, u, v)
    res = run_bass_kernel_spmd(nc, in_maps, list(range(NCORES)),
                               trace=os.environ.get("KBENCH_TRACE") == "1")
    LAST_RESULT = res

    def assemble(key):
        arr = np.stack([res.results[c][key] for c in range(NCORES)])
        return np.ascontiguousarray(
            arr.reshape(NCORES, 128, B, MC, R).transpose(2, 0, 3, 1, 4)
        ).reshape(B, M, R)

    return (assemble("u_out"), assemble("v_out"))
